# revision 4
# baseline (speedup 1.0000x reference)
"""Trainium2 Bass kernel for nn_BlockDecomposition (relational GNN message passing).

Reference computation:
    out[n] = keep[n] * (x[n] @ BD(blocks[-1]))                    (self loop)
           + sum_{directed edge e: tgt_e == n} w_e * (x[src_e] @ BD(blocks[et_e]))
where BD(.) embeds 32 4x4 blocks into a block-diagonal 128x128 matrix and the
edge list is symmetrized (each undirected edge appears in both directions).

Strategy (8 NeuronCores, no collectives):
  - Nodes are assigned to (core, block, tloc) by a host-side greedy balancer
    that equalizes per-(block, relation) edge counts ACROSS cores (the SPMD
    schedule is the max over cores, so balance = less padding). Blocks are
    ordered light, heavy...light so the pipeline head and tail are short.
  - Per half-block one dma_gather (GPSIMD SWDGE; cost is ~0.83ns/idx, pure
    per-index, so fine-grained calls cost nothing and overlap better) pulls
    the needed x rows from the HBM-resident fp16 x table into SBUF.
  - 16 edge relations form 4 supergroups of 4 "slots", each supergroup
    owning a [din, 512] fp32 PSUM bank. Each relation contributes
    max(1, gmax//128) dense 128-edge "full" tiles; ALL remainders of a block
    are concatenated into ONE global merged stream chopped into 128-edge
    tiles (one-hot column = global column over the 4 banks), so ceil-waste
    is paid once per block instead of once per supergroup. Merged tiles
    spanning a bank boundary issue one matmul per touched bank over the
    same one-hot.
  - Per tile: DVE builds a weighted one-hot in one fused tensor_scalar
    (is_equal, mult), fp16; PE scatter-matmul accumulates
    aggT[din, col] += xg[e, din] x OH[e, col] in fp32 PSUM. The PSUM zero
    region is 2KB (a whole bank), so only the first matmul into each bank
    carries start=True; later first-touches auto-zero.
  - Per supergroup: ACT copies the PSUM bank to SBUF fp16; PE transform
    matmuls out[n, dout] += agg[n, din] @ BD(W_r)[din, dout].
  - The SELF-LOOP skips the edge machinery: the host ships a keep-masked
    transposed xT table (per-core, permuted layout) and each block adds one
    transform matmul out += (keep*x)^T.T @ BD(W_16).

Numerics: gathered x, one-hots, and block weights are fp16; accumulation is
fp32 in PSUM. Host does index manipulation (balancing/sorting/padding),
dtype casts, boolean masking, and block-diagonal layout only.
"""

import os
import sys
import numpy as np

for _p in ("/opt/trn_rl_repo", "/root/.axon_site/_ro/trn_rl_repo"):
    if os.path.isdir(_p) and _p not in sys.path:
        sys.path.insert(0, _p)

import concourse.bass as bass
import concourse.bacc as bacc
import concourse.mybir as mybir
import concourse.tile as tile
from concourse.bass_utils import run_bass_kernel_spmd

# ----------------------------------------------------------------------------
# Problem constants (hardcoded per spec)
N_NODES = 10000
N_EDGES = 160000
NUM_REL = 16          # relations used by edges; blocks[16] is the self-loop
NUM_BLOCKS = 32
BLOCK_SIZE = 4
D = NUM_BLOCKS * BLOCK_SIZE   # 128
N_CORES = 8
NPC = N_NODES // N_CORES      # 1250 nodes per core
BLK = 128                     # node block size (partition dim of scatter)
NBLK = 10                     # blocks per core (9 x 128 + 1 x 98)
BLK_CAPS = [128] * 9 + [NPC - 9 * 128]   # last block holds 98 nodes
NRELS = NUM_REL               # edge relations (self-loop handled directly)
TILE_E = 128                  # edges per tile (matmul contraction dim)
LAST_BLK_WEIGHT = 2.5         # balancer bias: keep the last block light
MAX_WIN = 512                 # max one-hot window width (oh arena free dim)

F32 = mybir.dt.float32
F16 = mybir.dt.float16
I16 = mybir.dt.int16

_DEBUG_SIM = os.environ.get("KERNEL_USE_CORESIM", "0") == "1"

SUPERGROUPS = [list(range(4 * g, 4 * g + 4)) for g in range(4)]


# ----------------------------------------------------------------------------
# Host-side preprocessing: integer index manipulation only.

def _balance_nodes(tgt2, et2):
    """Assign nodes to (core, block, tloc) minimizing the padded schedule.

    The SPMD schedule length per (block, rel) cell is max over cores of the
    cell's edge count, so we greedily place nodes (with their per-rel incoming
    edge count vectors) into the 80 (core, block) bins to minimize
    sum_{b,r} max_c cnt[c,b,r]. Returns (node_core, node_blk, node_tloc).
    """
    v = np.zeros((N_NODES, NRELS), dtype=np.int32)
    np.add.at(v, (tgt2, et2), 1)
    deg = v.sum(axis=1)

    order = np.argsort(-deg, kind="stable")
    cnt = np.zeros((N_CORES, NBLK, NRELS), dtype=np.int64)
    space = np.tile(np.array(BLK_CAPS, dtype=np.int64), (N_CORES, 1))
    # near-flat load; the pipeline model (finish >= max_k gather_end(k) +
    # pe_work(k..)) favors a light head and an ascending ramp, arranged by
    # the relabel below
    wgt = np.array([1.12, 1.0, 1.0, 1.0, 1.0, 1.0, 1.0, 1.0, 1.0, 1.0])

    node_core = np.zeros(N_NODES, dtype=np.int32)
    node_blk = np.zeros(N_NODES, dtype=np.int32)

    M = cnt.max(axis=0)  # [NBLK, NRELS] current per-cell max
    for n in order:
        vn = v[n]
        delta = np.maximum(cnt + vn[None, None, :] - M[None, :, :], 0).sum(axis=2)
        # small pressure toward equal per-block schedule lengths (flat tile
        # distribution) on top of the exact padding delta
        cost = (
            delta * wgt[None, :]
            + 0.03 * M.sum(axis=1)[None, :]
            + 1e-4 * (128 - space)
        )
        cost[space <= 0] = np.inf
        c, b = np.unravel_index(np.argmin(cost), cost.shape)
        node_core[n] = c
        node_blk[n] = b
        cnt[c, b] += vn
        space[c, b] -= 1
        M[b] = np.maximum(M[b], cnt[c, b])

    # relabel the equal-cap blocks 0..8 so loads ascend (light head; late
    # tiles only cost PE 69ns each while early tiles delay the whole
    # gather-paced pipeline by 107ns each); the 98-cap block stays last
    load = cnt.max(axis=0).sum(axis=1)[: NBLK - 1]
    relabel = np.empty(NBLK, dtype=np.int32)
    relabel[np.argsort(load, kind="stable")] = np.arange(NBLK - 1)
    relabel[NBLK - 1] = NBLK - 1
    node_blk = relabel[node_blk]

    node_tloc = np.zeros(N_NODES, dtype=np.int32)
    fill = np.zeros((N_CORES, NBLK), dtype=np.int32)
    for n in range(N_NODES):
        c, b = node_core[n], node_blk[n]
        node_tloc[n] = fill[c, b]
        fill[c, b] += 1
    return node_core, node_blk, node_tloc


def _build_schedule(cnt):
    """Static tile schedule shared by all cores.

    cnt: [C, NBLK, NRELS] per-core (block, rel) edge counts.

    Per block: 4 supergroups of <=4 relation "slots", each owning a
    [din, 512] PSUM bank. Global one-hot column of (sg, slot j) is
    coloff[sg] + 128*j (coloff compacted over used slots). Each present
    relation gets max(1, gmax//128) full tiles (one-hot window = its slot);
    all remainders form ONE global stream chopped into <=128-edge merged
    tiles with window width <= MAX_WIN (early-cut if a slot would exceed).

    Returns (sched, Ttot): sched[b] = {
      "sgs": [ {sgi, rels, slots, full, rem, used, coloff} ],
      "tiles": [ per tile dict:
          kind "full":  {sgi, r, c0, c1, start, local_iota}
          kind "merged":{segs: [(sgi, c0, c1)], c0, c1}  # global window
        ],
      "stops": {sgi: tile_index_of_last_touch}
    }"""
    gmax = cnt.max(axis=0)  # [NBLK, NRELS]
    sched = []
    Ttot = 0
    for b in range(NBLK):
        sgs = []
        coloff = 0
        for sgi, rels_all in enumerate(SUPERGROUPS):
            rels = [r for r in rels_all if gmax[b, r] > 0]
            if not rels:
                continue
            slots = {r: j for j, r in enumerate(rels)}
            full = {}
            rem = {}
            for r in rels:
                g = int(gmax[b, r])
                f = g // TILE_E
                if f == 0:
                    f, rr = 1, 0
                else:
                    rr = g % TILE_E
                full[r], rem[r] = f, rr
            sgs.append(
                {
                    "sgi": sgi,
                    "rels": rels,
                    "slots": slots,
                    "full": full,
                    "rem": rem,
                    "used": len(rels) * BLK,
                    "coloff": coloff,
                }
            )
            coloff += len(rels) * BLK

        tiles = []
        last_touch = {}
        colrel = {}  # (sgi, col_base) -> rel
        cur = []   # slots in current merged tile: (sgi, col_base, take)
        cur_e = 0

        def _flush():
            nonlocal cur, cur_e
            if not cur:
                return
            segs = []
            for sgi_, cb, take in cur:
                if segs and segs[-1][0] == sgi_:
                    lo = min(segs[-1][1], cb)
                    hi = max(segs[-1][2], cb + BLK)
                    segs[-1] = (sgi_, lo, hi)
                else:
                    segs.append((sgi_, cb, cb + BLK))
            c0 = min(s[1] for s in segs)
            c1 = max(s[2] for s in segs)
            assert c1 - c0 <= MAX_WIN
            tiles.append(
                {
                    "kind": "merged",
                    "segs": segs,
                    "c0": c0,
                    "c1": c1,
                    "packed": list(cur),
                }
            )
            for sgi_, _, _ in segs:
                last_touch[sgi_] = len(tiles) - 1
            cur, cur_e = [], 0

        def _emit_fulls(sg):
            first = True
            for r in sg["rels"]:
                j = sg["slots"][r]
                c0 = sg["coloff"] + j * BLK
                colrel[(sg["sgi"], c0)] = r
                for t in range(sg["full"][r]):
                    tiles.append(
                        {
                            "kind": "full",
                            "sgi": sg["sgi"],
                            "r": r,
                            "c0": c0,
                            "c1": c0 + BLK,
                            "start": first,
                        }
                    )
                    last_touch[sg["sgi"]] = len(tiles) - 1
                    first = False

        def _emit_merged(stream):
            nonlocal cur, cur_e
            for sgi_, cb, rlen in stream:
                # nearly-full tiles flush before opening a new slot: pad rows
                # are free (matmul cost is window width, not edge count)
                # while widening the window costs DVE+PE on every tile
                if cur_e >= 112:
                    _flush()
                off = 0
                while off < rlen:
                    win_lo = min([c for _, c, _ in cur], default=cb)
                    if cur_e >= TILE_E or (cb + BLK) - win_lo > MAX_WIN:
                        _flush()
                    take = min(rlen - off, TILE_E - cur_e)
                    cur.append((sgi_, cb, take))
                    cur_e += take
                    off += take
            _flush()

        def _sg_stream(sg):
            return [
                (sg["sgi"], sg["coloff"] + sg["slots"][r] * BLK, sg["rem"][r])
                for r in sg["rels"]
                if sg["rem"][r] > 0
            ]

        sg_ranges = []
        if b == NBLK - 1:
            # last block: self-contained per-supergroup sections so the drain
            # only waits on the final section's chain, and gathers can be
            # issued per section
            for sg in sgs:
                t0 = len(tiles)
                _emit_fulls(sg)
                _emit_merged(_sg_stream(sg))
                sg_ranges.append((t0, len(tiles)))
        else:
            for sg in sgs:
                _emit_fulls(sg)
            _emit_merged([s for sg in sgs for s in _sg_stream(sg)])

        sched.append(
            {
                "sgs": sgs,
                "tiles": tiles,
                "stops": last_touch,
                "colrel": colrel,
                "sg_ranges": sg_ranges,
            }
        )
        Ttot += len(tiles)
    return sched, Ttot


def _preprocess(x, node_keep_mask, source, target, edge_type, edge_weights):
    """Build the balanced permutation and per-core padded tile schedule."""
    src = np.asarray(source).astype(np.int64)
    tgt = np.asarray(target).astype(np.int64)
    et = np.asarray(edge_type).astype(np.int64)
    ew = np.asarray(edge_weights).astype(np.float32)

    srcA = np.concatenate([src, tgt])
    tgtA = np.concatenate([tgt, src])
    etA = np.concatenate([et, et])
    ewA = np.concatenate([ew, ew])

    node_core, node_blk, node_tloc = _balance_nodes(tgtA, etA)

    core = node_core[tgtA]
    blk = node_blk[tgtA]
    tloc = node_tloc[tgtA].astype(np.float32)

    order = np.lexsort((etA, blk, core))
    srcS = srcA[order].astype(np.int16)
    tlocS = tloc[order]
    ewS = ewA[order]

    key = (core * NBLK + blk) * NRELS + etA
    cnt2 = np.bincount(key, minlength=N_CORES * NBLK * NRELS).reshape(
        N_CORES, NBLK, NRELS
    )
    starts = np.concatenate([[0], np.cumsum(cnt2.reshape(-1))]).astype(np.int64)

    sched, Ttot = _build_schedule(cnt2)

    src_pad = np.zeros((N_CORES, Ttot * TILE_E), dtype=np.int16)
    tloc_pad = np.zeros((N_CORES, Ttot * TILE_E), dtype=np.float32)
    w_pad = np.zeros((N_CORES, Ttot * TILE_E), dtype=np.float32)

    # slot-region offsets within each block's tile stream (shared by cores):
    # walk tiles in order; full tiles expose a (sgi, r) full region;
    # merged tiles expose per-(sgi, col_base) spans in stream order.
    for c in range(N_CORES):
        pos = 0
        for b in range(NBLK):
            blkrec = sched[b]
            # per-rel edge list split into full part and remainder part
            ecur = {}
            for sg in blkrec["sgs"]:
                for r in sg["rels"]:
                    gi = (c * NBLK + b) * NRELS + r
                    s0 = int(starts[gi])
                    n = int(cnt2[c, b, r])
                    nf = min(n, sg["full"][r] * TILE_E)
                    ecur[r] = (s0, nf, s0 + nf, n - nf)
                    assert n - nf <= sg["rem"][r]
            fpos = {}  # per-rel cursor into its full region
            mpos = {}  # per-rel cursor into its remainder
            for tl in blkrec["tiles"]:
                # tloc is stored RELATIVE to the tile's one-hot window start
                # (tl["c0"]) so the device iota table is only 512 wide
                if tl["kind"] == "full":
                    r = tl["r"]
                    s0, nf, _, _ = ecur[r]
                    done = fpos.get(r, 0)
                    take = min(TILE_E, max(0, nf - done))
                    if take > 0:
                        src_pad[c, pos : pos + take] = srcS[s0 + done : s0 + done + take]
                        tloc_pad[c, pos : pos + take] = tlocS[s0 + done : s0 + done + take]
                        w_pad[c, pos : pos + take] = ewS[s0 + done : s0 + done + take]
                    fpos[r] = done + take
                else:
                    tpos = pos
                    for sgi_, cb, take in tl["packed"]:
                        r = blkrec["colrel"][(sgi_, cb)]
                        _, _, m0, mrem = ecur[r]
                        done = mpos.get(r, 0)
                        use = min(take, max(0, mrem - done))
                        if use > 0:
                            src_pad[c, tpos : tpos + use] = srcS[m0 + done : m0 + done + use]
                            tloc_pad[c, tpos : tpos + use] = (
                                tlocS[m0 + done : m0 + done + use]
                                + float(cb - tl["c0"])
                            )
                            w_pad[c, tpos : tpos + use] = ewS[m0 + done : m0 + done + use]
                        mpos[r] = done + use
                        tpos += take
                pos += TILE_E
        assert pos == Ttot * TILE_E, (pos, Ttot * TILE_E)
    return {
        "sched": sched,
        "Ttot": Ttot,
        "src_pad": src_pad,
        "tloc_pad": tloc_pad,
        "w_pad": w_pad,
        "node_core": node_core,
        "node_blk": node_blk,
        "node_tloc": node_tloc,
    }


def _make_bdw(blocks):
    """blocks [17, 32, 4, 4] -> dense block-diagonal lhsT layout [128, 17*128]."""
    blocks = np.asarray(blocks).astype(np.float32)
    bdw = np.zeros((D, (NUM_REL + 1) * D), dtype=np.float32)
    for r in range(NUM_REL + 1):
        for b in range(NUM_BLOCKS):
            bdw[
                b * BLOCK_SIZE : (b + 1) * BLOCK_SIZE,
                r * D + b * BLOCK_SIZE : r * D + (b + 1) * BLOCK_SIZE,
            ] = blocks[r, b]
    return bdw


def _tiles_per_block(sched):
    return [len(blk["tiles"]) for blk in sched]


def _gather_calls(sched):
    """(block, tile_offset, ntiles) per dma_gather call: block 0 split
    [4, rest] for a fast head; other blocks split in halves for overlap;
    the last block gets one call per supergroup section so its per-sg
    compute chains drain as the gathers land."""
    tiles_per_block = _tiles_per_block(sched)
    calls = []
    nblk = len(tiles_per_block)
    for b, tb in enumerate(tiles_per_block):
        if tb == 0:
            continue
        if b == nblk - 1 and sched[b]["sg_ranges"]:
            for t0, t1 in sched[b]["sg_ranges"]:
                if t1 > t0:
                    calls.append((b, t0, t1 - t0))
            continue
        h1 = min(4, tb) if b == 0 else (tb + 1) // 2
        calls.append((b, 0, h1))
        if tb - h1 > 0:
            calls.append((b, h1, tb - h1))
    return calls


def _wrap_idxs(src_pad_core, sched):
    """Wrapped gather-index layout, one segment per gather call."""
    tiles_per_block = _tiles_per_block(sched)
    bounds = np.concatenate([[0], np.cumsum(tiles_per_block)]) * TILE_E
    cols = []
    for b, toff, ntl in _gather_calls(sched):
        off = int(bounds[b]) + toff * TILE_E
        ni = ntl * TILE_E
        seg = src_pad_core[off : off + ni]
        wrapped = seg.reshape(ni // 16, 16).T
        cols.append(np.tile(wrapped, (8, 1)))
    return np.ascontiguousarray(np.concatenate(cols, axis=1))


# ----------------------------------------------------------------------------
# Bass kernel builder (one SPMD program for all cores)

def _build_nc(prep):
    sched = prep["sched"]
    Ttot = prep["Ttot"]
    tiles_per_block = _tiles_per_block(sched)
    calls = _gather_calls(sched)

    nc = bacc.Bacc("TRN2", target_bir_lowering=False, debug=False, num_devices=N_CORES)

    tb0 = tiles_per_block[0]
    Tr = Ttot - tb0  # tiles in blocks 1..9
    # head1: block-0 [tloc|w] (f32 as 2 i16) | iota512 | block-0 gather idxs
    # — meta first so the first DMA chunk unblocks DVE and Pool together
    C1 = 4 * tb0 + 512 + 8 * tb0
    S0 = 4 * tb0 + 512  # start of block-0 gather idxs
    # head2: remaining gather idxs | remaining per-block [tloc|w]
    C2 = 8 * Tr + 4 * Tr

    x_d = nc.declare_dram_parameter("x16", [N_NODES, D], F16, isOutput=False)
    head1_d = nc.declare_dram_parameter("head1", [128, C1], I16, isOutput=False)
    head2_d = nc.declare_dram_parameter("head2", [128, C2], I16, isOutput=False)
    bdw_d = nc.declare_dram_parameter("bdw16", [128, (NUM_REL + 1) * D], F16, isOutput=False)
    xtm_d = nc.declare_dram_parameter("xtm16", [128, NBLK * BLK], F16, isOutput=False)
    out_d = nc.declare_dram_parameter("out", [NBLK * BLK, D], F32, isOutput=True)

    # per-block [tloc|w] column offsets within the f32 view of head2
    moff2 = np.concatenate(
        [[0], np.cumsum([2 * t for t in tiles_per_block[1:]])]
    )

    with tile.TileContext(nc) as tc:
        with (
            tc.tile_pool(name="const", bufs=1) as const_pool,
            tc.tile_pool(name="xg", bufs=3) as xg_pool,
            tc.tile_pool(name="oh", bufs=2) as oh_pool,
            tc.tile_pool(name="aggsb", bufs=6) as aggsb_pool,
            tc.tile_pool(name="outsb", bufs=3) as outsb_pool,
            tc.tile_pool(name="psA", bufs=5, space=bass.MemorySpace.PSUM) as psA_pool,
            tc.tile_pool(name="psO", bufs=3, space=bass.MemorySpace.PSUM) as psO_pool,
        ):
            # --- constants. SP: head1 (everything block 0 + iota, one fast
            # DMA), then head2. ACT (behind its act-table load): xtm, bdw.
            head1_sb = const_pool.tile([128, C1], I16, tag="head1")
            # first chunk = block-0 meta + iota + the 4-tile head gather's
            # indices: one DMA unblocks the DVE one-hot stream AND the first
            # gather at ~1.9us
            nc.sync.dma_start(head1_sb[:, 0 : S0 + 32], head1_d[:, 0 : S0 + 32])
            nc.sync.dma_start(head1_sb[:, S0 + 32 :], head1_d[:, S0 + 32 :])
            head2_sb = const_pool.tile([128, C2], I16, tag="head2")
            nc.sync.dma_start(head2_sb[:], head2_d[:, :])
            xtm_sb = const_pool.tile([128, NBLK * BLK], F16, tag="xtm")
            nc.scalar.dma_start(xtm_sb[:], xtm_d[:, :])
            bdw_sb = const_pool.tile([128, (NUM_REL + 1) * D], F16, tag="bdw")
            nc.scalar.dma_start(bdw_sb[:], bdw_d[:, :])

            metaf0 = head1_sb[:, 0 : 4 * tb0].bitcast(F32)
            iota_sb = head1_sb[:, 4 * tb0 : S0].bitcast(F16)
            metaf2 = head2_sb[:, 8 * Tr :].bitcast(F32)

            # --- gathers (per half-block; cost is per-index, calls are free)
            xg_tiles = {}
            scol = 0
            prev_head1 = True
            for b, toff, ntl in calls:
                if b >= 1 and prev_head1:
                    scol = 0  # switch from head1 to head2 index region
                    prev_head1 = False
                if b not in xg_tiles:
                    xg_tiles[b] = xg_pool.tile(
                        [128, tiles_per_block[b], D], F16, name="xg", tag="xg"
                    )
                xg = xg_tiles[b]
                if b == 0:
                    idx_ap = head1_sb[:, S0 + scol : S0 + scol + ntl * 8]
                else:
                    idx_ap = head2_sb[:, scol : scol + ntl * 8]
                nc.gpsimd.dma_gather(
                    out_ap=xg[:, toff : toff + ntl, :],
                    in_ap=x_d[:, :],
                    idxs_ap=idx_ap,
                    num_idxs=ntl * TILE_E,
                    num_idxs_reg=ntl * TILE_E,
                    elem_size=D,
                    single_packet=False,
                )
                scol += ntl * 8

            # --- per-block compute
            for b in range(NBLK):
                blkrec = sched[b]
                tb = tiles_per_block[b]
                if tb == 0:
                    continue
                xg = xg_tiles[b]
                if b == 0:
                    tloc_sb = metaf0[:, 0:tb]
                    w_sb = metaf0[:, tb : 2 * tb]
                else:
                    o = int(moff2[b - 1])
                    tloc_sb = metaf2[:, o : o + tb]
                    w_sb = metaf2[:, o + tb : o + 2 * tb]

                out_ps = psO_pool.tile([BLK, D], F32, tag="outps")
                n_transforms = 1 + sum(len(sg["rels"]) for sg in blkrec["sgs"])

                oh_blk = oh_pool.tile([128, tb, MAX_WIN], F16, tag="oh")
                agg_ps = {}
                for sg in blkrec["sgs"]:
                    agg_ps[sg["sgi"]] = psA_pool.tile(
                        [D, 4 * BLK], F32, name="aggps", tag="aggps"
                    )
                sg_by_i = {sg["sgi"]: sg for sg in blkrec["sgs"]}

                for bt, tl in enumerate(blkrec["tiles"]):
                    c0, c1 = tl["c0"], tl["c1"]
                    oh = oh_blk[:, bt, :]
                    # tloc values are tile-window-relative; iota is [0..512)
                    if tl["kind"] == "full":
                        iota_ap = iota_sb[:, 0:BLK]
                        ohw = oh[:, 0:BLK]
                    else:
                        iota_ap = iota_sb[:, 0 : c1 - c0]
                        ohw = oh[:, 0 : c1 - c0]
                    nc.vector.tensor_scalar(
                        ohw,
                        iota_ap,
                        tloc_sb[:, bt : bt + 1],
                        w_sb[:, bt : bt + 1],
                        mybir.AluOpType.is_equal,
                        mybir.AluOpType.mult,
                    )
                    if tl["kind"] == "full":
                        sg = sg_by_i[tl["sgi"]]
                        a0 = c0 - sg["coloff"]
                        nc.tensor.matmul(
                            agg_ps[tl["sgi"]][:, a0 : a0 + BLK],
                            xg[:, bt, :],
                            ohw,
                            start=tl["start"],
                            stop=(blkrec["stops"][tl["sgi"]] == bt),
                            skip_group_check=True,
                        )
                    else:
                        for sgi_, s0, s1 in tl["segs"]:
                            sg = sg_by_i[sgi_]
                            a0 = s0 - sg["coloff"]
                            nc.tensor.matmul(
                                agg_ps[sgi_][:, a0 : a0 + (s1 - s0)],
                                xg[:, bt, :],
                                oh[:, s0 - c0 : s1 - c0],
                                start=False,
                                stop=(blkrec["stops"][sgi_] == bt),
                                skip_group_check=True,
                            )
                # phase 2: self-loop transform (first out_ps matmul in
                # program order so its start=True resets the bank), then
                # PSUM->SBUF copies + per-relation transform matmuls
                ti = 0
                nc.tensor.matmul(
                    out_ps[:],
                    xtm_sb[:, b * BLK : (b + 1) * BLK],
                    bdw_sb[:, NUM_REL * D : (NUM_REL + 1) * D],
                    start=True,
                    stop=(n_transforms == 1),
                )
                ti += 1
                for sgn, sg in enumerate(blkrec["sgs"]):
                    used = sg["used"]
                    agg_sb = aggsb_pool.tile([D, 4 * BLK], F16, tag="aggsb")
                    # in the drain phase DVE is idle — let it take alternate
                    # copies so PE's transform chain isn't ACT-throughput-bound
                    if b >= NBLK - 1 and sgn % 2 == 1:
                        nc.vector.tensor_copy(
                            agg_sb[:, :used], agg_ps[sg["sgi"]][:, :used]
                        )
                    else:
                        nc.scalar.copy(agg_sb[:, :used], agg_ps[sg["sgi"]][:, :used])
                    for r in sg["rels"]:
                        j = sg["slots"][r]
                        nc.tensor.matmul(
                            out_ps[:],
                            agg_sb[:, j * BLK : (j + 1) * BLK],
                            bdw_sb[:, r * D : (r + 1) * D],
                            start=False,
                            stop=(ti == n_transforms - 1),
                        )
                        ti += 1
                out_sb = outsb_pool.tile([BLK, D], F32, tag="outsb")
                if b >= NBLK - 1:
                    nc.vector.tensor_copy(out_sb[:], out_ps[:])
                else:
                    nc.scalar.copy(out_sb[:], out_ps[:])
                nc.sync.dma_start(out_d[b * BLK : (b + 1) * BLK, :], out_sb[:])
    nc.compile()
    return nc


# ----------------------------------------------------------------------------

def _make_in_maps(x, prep, node_keep_mask, blocks):
    sched = prep["sched"]
    tpb = _tiles_per_block(sched)
    tb0 = tpb[0]
    bdw = _make_bdw(blocks)
    iota16 = np.ascontiguousarray(
        np.tile(np.arange(512, dtype=np.float16)[None, :], (128, 1))
    )

    x16 = x.astype(np.float16)
    keep = np.asarray(node_keep_mask).astype(bool)
    xm = np.where(keep[:, None], x, 0.0).astype(np.float16)

    node_core = prep["node_core"]
    node_blk = prep["node_blk"]
    node_tloc = prep["node_tloc"]
    slot = node_blk.astype(np.int64) * BLK + node_tloc

    bdw16 = np.ascontiguousarray(bdw.astype(np.float16))

    in_maps = []
    for c in range(N_CORES):
        mcols = []
        off = 0
        for b in range(NBLK):
            nb_ = tpb[b] * TILE_E
            mcols.append(prep["tloc_pad"][c][off : off + nb_].reshape(tpb[b], 128).T)
            mcols.append(prep["w_pad"][c][off : off + nb_].reshape(tpb[b], 128).T)
            off += nb_
        metaf0 = np.ascontiguousarray(np.concatenate(mcols[:2], axis=1))
        metaf2 = np.ascontiguousarray(np.concatenate(mcols[2:], axis=1))

        srcidx = _wrap_idxs(prep["src_pad"][c], sched)  # [128, Ttot*8] i16
        head1 = np.ascontiguousarray(
            np.concatenate(
                [
                    metaf0.view(np.int16),
                    iota16.view(np.int16),
                    srcidx[:, 0 : 8 * tb0],
                ],
                axis=1,
            )
        )
        head2 = np.ascontiguousarray(
            np.concatenate(
                [srcidx[:, 8 * tb0 :], metaf2.view(np.int16)], axis=1
            )
        )

        xtm = np.zeros((128, NBLK * BLK), dtype=np.float16)
        mine = node_core == c
        xtm[:, slot[mine]] = xm[mine].T

        in_maps.append(
            {
                "x16": x16,
                "head1": head1,
                "head2": head2,
                "bdw16": bdw16,
                "xtm16": xtm,
            }
        )
    return in_maps


def kernel(x, node_keep_mask, source, target, edge_type, edge_weights, blocks):
    global LAST_NC, LAST_IN_MAPS
    x = np.ascontiguousarray(np.asarray(x), dtype=np.float32)
    prep = _preprocess(x, node_keep_mask, source, target, edge_type, edge_weights)
    in_maps = _make_in_maps(x, prep, node_keep_mask, blocks)
    nc = _build_nc(prep)
    LAST_NC, LAST_IN_MAPS = nc, in_maps

    node_core = prep["node_core"]
    slot = prep["node_blk"].astype(np.int64) * BLK + prep["node_tloc"]

    if _DEBUG_SIM:
        from concourse.bass_interp import CoreSim

        outs = []
        for c in range(N_CORES):
            sim = CoreSim(nc)
            for k, v in in_maps[c].items():
                sim.tensor(k)[:] = v
            sim.simulate()
            outs.append(np.array(sim.tensor("out")))
        out_full = np.zeros((N_NODES, D), dtype=np.float32)
        for c in range(N_CORES):
            mine = np.where(node_core == c)[0]
            out_full[mine] = outs[c][slot[mine]]
        return out_full

    trace = os.environ.get("KERNEL_TRACE", "0") == "1"
    res = run_bass_kernel_spmd(
        nc, in_maps, core_ids=list(range(N_CORES)), trace=trace
    )
    global LAST_EXEC_TIME_NS
    LAST_EXEC_TIME_NS = res.exec_time_ns
    out_full = np.zeros((N_NODES, D), dtype=np.float32)
    for c in range(N_CORES):
        mine = np.where(node_core == c)[0]
        out_full[mine] = np.asarray(res.results[c]["out"])[slot[mine]]
    return out_full


LAST_EXEC_TIME_NS = None
LAST_NC = None
LAST_IN_MAPS = None
